# revision 1
# baseline (speedup 1.0000x reference)
"""Bahdanau-attention kernel for Trainium2, data-parallel over 8 NeuronCores.

Contract: kernel(**inputs) takes FULL unsharded numpy inputs and returns the
FULL output (context [4096,32] f32, att_weights [4096,196] f32), matching the
reference nn.Module. Batch (4096) is sharded 512-per-core across the 8 cores;
the tiny weight matrices are replicated.
"""

import numpy as np
import jax
import jax.numpy as jnp
from functools import partial

B, C, H, W = 4096, 32, 14, 14
P = H * W            # 196 spatial positions
ENC_DIM = 512
ATT_DIM = 128
N_CORES = 8

_compiled = {}


def _attention_shard(img_features, prev_hidden, W_w, W_b, U_w, U_b, v_w, v_b,
                     fb_w, fb_b):
    # img_features: [b, C, H, W] -> [b, P, C]
    b = img_features.shape[0]
    img_parts = img_features.reshape(b, C, P).transpose(0, 2, 1)
    W_s = jnp.einsum('bpc,ac->bpa', img_parts, W_w) + W_b        # [b, P, 128]
    U_h = prev_hidden @ U_w.T + U_b                              # [b, 128]
    att = jnp.tanh(W_s + U_h[:, None, :])                        # [b, P, 128]
    score = jnp.einsum('bpa,a->bp', att, v_w[0]) + v_b[0]        # [b, P]
    att_weights = jax.nn.softmax(score, axis=1)                  # [b, P]
    context = jnp.einsum('bpc,bp->bc', img_parts, att_weights)   # [b, C]
    beta = jax.nn.sigmoid(prev_hidden @ fb_w.T + fb_b)           # [b, C]
    return context * beta, att_weights


def _get_pmapped():
    if 'fn' not in _compiled:
        devs = jax.devices()[:N_CORES]
        _compiled['fn'] = jax.pmap(
            _attention_shard,
            axis_name='dp',
            in_axes=(0, 0, None, None, None, None, None, None, None, None),
            devices=devs,
        )
    return _compiled['fn']


def kernel(img_features, prev_hidden, W_w, W_b, U_w, U_b, v_w, v_b, fb_w, fb_b):
    img_features = np.asarray(img_features, dtype=np.float32)
    prev_hidden = np.asarray(prev_hidden, dtype=np.float32)

    shard = B // N_CORES
    img_sh = img_features.reshape(N_CORES, shard, C, H, W)
    hid_sh = prev_hidden.reshape(N_CORES, shard, ENC_DIM)

    fn = _get_pmapped()
    ctx, att = fn(img_sh, hid_sh,
                  jnp.asarray(W_w), jnp.asarray(W_b),
                  jnp.asarray(U_w), jnp.asarray(U_b),
                  jnp.asarray(v_w), jnp.asarray(v_b),
                  jnp.asarray(fb_w), jnp.asarray(fb_b))

    context = np.asarray(ctx).reshape(B, C).astype(np.float32)
    att_weights = np.asarray(att).reshape(B, P).astype(np.float32)
    return context, att_weights
